# revision 6
# baseline (speedup 1.0000x reference)
"""BinConv2dEval Trainium2 kernel (fp8 DoubleRow, packed-65 layout).

y = conv2d(x, W, stride 1, pad 1) + bias ; out = (round(y) * sign >= 0) ? 1 : 0

All values are integers (x in {0,1}, W in {-1,0,1}, bias integer), so round()
is a no-op and everything is exact in fp8e4 matmuls with fp32 PSUM
accumulation. Folding: with s = sign[c] in {+-1},
    (conv + bias) * s >= 0   <=>   conv(x, s*W) >= -s*bias
so host-side we fold sign into the (still ternary) weights and compare each
output channel against a per-channel threshold with one DVE is_ge op.

Sharding: data-parallel over batch N=32 -> 4 images per core on 8 cores.
Weights/bias/sign are tiny and replicated.

Layout: width-65 rows ([64 data][0]) with SHARED zero rows between images:
row stream = [z, img0 r0..63, z, img1 r0..63, z, img2 ..., z, img3 ..., z]
= 261 rows x 65 = 16965 elems per partition. The single trailing zero col
doubles as both x(r,-1) of the next row and x(r,64) of its own, and each
separator zero row is both bottom pad of img i and top pad of img i+1. The
conv's 9 taps become pure element offsets (kh-1)*65 + (kw-1), and the
4-image output region is ONE contiguous stream of 16834 positions per cout
half (junk at stored col 64 and in separator rows; host strips ~2.7%).

Taps pair into fp8 DoubleRow matmuls (2 MACs/cell/cycle, contraction 256):
(kh=0,kw)+(kh=1,kw) at stride 65 for kw in 0..2, (kh=2,kw=0)+(kh=2,kw=2) at
stride 2; only (kh=2,kw=1) is a normal-rate matmul -> 5 passes per tile.
Per half: 33 PSUM tiles (32x512 + 450), weight-stationary subgroups of <=4
tiles; one DVE tensor_scalar(is_ge) per tile drains PSUM -> SBUF as 0/1
**fp8** (4x less output DMA than fp32; host upcasts). Half1 ends with 2/2/1
tile subgroups so the final DVE drains hide under the last matmuls.

Head: a few zero-weight warm-up matmuls sized to the DMA doorbell latency
(~1.7us) + first-chunk transfer, so the PE HAM clock-gate releases around
the time real compute starts; input x is chunked so tile 0 lands first.
"""

import numpy as np
import ml_dtypes

N, CIN, H, W = 32, 128, 64, 64
COUT, KH, KW = 256, 3, 3
N_CORES = 8
IMGS = N // N_CORES          # 4 images per core
WS = W + 1                   # 65: stored row width ([64 data][0])
ROWS_T = 1 + IMGS * (H + 1)  # 261 stored rows (shared separators)
XD = ROWS_T * WS             # 16965 elements per partition
GF = 16                      # guard zeros before the image block
XTOT = GF + XD
OBASE = WS                   # first output position in the stream (img0 r0 c0)
OUT_N = (1 + IMGS * (H + 1) - 2) * WS + (W - 1) - OBASE + 1  # 16834
NB = 512                     # full PSUM tile free dim (one bank)
NBS = [NB] * 32 + [OUT_N - 32 * NB]   # 33 tiles: 32x512 + 450
NHALF = COUT // 128          # 2 cout halves
NPAIR = 4                    # DoubleRow tap pairs per pass group
FP8 = ml_dtypes.float8_e4m3  # TRN float8e4; {-1,0,1} and {0,1} are exact
NWARM = 6                    # zero-weight PE warm-up matmuls (N=512, cold)

# weight-stationary spans (start tile, n tiles); half1 tapers so the last
# DVE drains overlap the final matmuls instead of trailing them
SG_HALF = (
    tuple((s, 4) for s in range(0, 32, 4)) + ((32, 1),),
    tuple((s, 4) for s in range(0, 28, 4)) + ((28, 2), (30, 2), (32, 1)),
)

# input x chunk boundaries (elements per partition): tiles0-3 | tiles4-15 |
# rest. Tile t reads [OBASE + 512t - 66, OBASE + 512t + NBS + 65]. All x
# chunks ride ONE ring (sync) in need order — the 16 DMA engines are shared
# between rings, so a big low-priority chunk on the other ring would starve
# the critical first tiles.
XCH = (2210, 8450, XD)

_CACHE = {}
LAST_RESULT = None           # BassKernelResults of the last run (for profiling)


def _build():
    import concourse.bass as bass
    import concourse.mybir as mybir
    from concourse import bacc
    from concourse.tile import TileContext

    dt = mybir.dt
    nc = bacc.Bacc()
    xp = nc.dram_tensor("xp", [128, XD], dt.float8e4, kind="ExternalInput")
    # pair weights: [cin, pair, 2, cout] flattened; pairs 0..2 = (kh0,kh1) per
    # kw, pair 3 = ((kh2,kw0),(kh2,kw2))
    wtp = nc.dram_tensor(
        "wtp", [128, NPAIR * 2 * COUT], dt.float8e4, kind="ExternalInput"
    )
    # the lone single tap (kh2,kw1): [cin, cout]
    wts = nc.dram_tensor("wts", [128, COUT], dt.float8e4, kind="ExternalInput")
    th = nc.dram_tensor("th", [128, NHALF], dt.float32, kind="ExternalInput")
    out = nc.dram_tensor(
        "out", [NHALF, 128, OUT_N], dt.float8e4, kind="ExternalOutput"
    )

    DR = mybir.MatmulPerfMode.DoubleRow
    # (pair rhs offset, pair stride) per DoubleRow pair index
    PAIR_GEOM = [(-66, WS), (-65, WS), (-64, WS), (64, 2)]
    SINGLE_OFF = WS  # (kh2, kw1)

    with TileContext(nc) as tc:
        with (
            tc.tile_pool(name="const", bufs=1) as cpool,
            tc.tile_pool(name="xin", bufs=1) as xpool,
            tc.tile_pool(name="psum", bufs=8, space="PSUM") as ppool,
            tc.tile_pool(name="outb", bufs=5) as opool,
        ):
            # warm-up operands first in gpsimd order so dummies start early
            wz_t = cpool.tile([128, 128], dt.float8e4, tag="wz")
            nc.gpsimd.memset(wz_t[:], 0)
            xz_t = cpool.tile([128, NB], dt.float8e4, tag="xz")
            nc.gpsimd.memset(xz_t[:], 0)

            xs_t = xpool.tile([128, XTOT], dt.float8e4, tag="xs")
            xs = xs_t[:]
            # front guard (junk reads at o=OBASE-66 must not hit fp8 NaNs)
            nc.gpsimd.memset(xs[:, :GF], 0)

            # sync HWDGE queue: weights then x chunks, strictly in need order
            wtp_t = cpool.tile([128, NPAIR * 2 * COUT], dt.float8e4, tag="wtp")
            nc.sync.dma_start(out=wtp_t[:], in_=wtp[:])
            lo = 0
            for hi in XCH:
                nc.sync.dma_start(out=xs[:, GF + lo : GF + hi], in_=xp[:, lo:hi])
                lo = hi
            # scalar HWDGE queue: only the tiny single-tap weights + thresholds
            wts_t = cpool.tile([128, COUT], dt.float8e4, tag="wts")
            nc.scalar.dma_start(out=wts_t[:], in_=wts[:])
            th_t = cpool.tile([128, NHALF], dt.float32, tag="th")
            nc.scalar.dma_start(out=th_t[:], in_=th[:])

            # Warm the PE clock (HAM un-throttle needs ~3.4us of sustained
            # activity) with zero-weight matmuls on a zeroed scratch tile
            # while the input DMA doorbell+transfer is still in flight.
            pd = ppool.tile([128, NB], dt.float32, tag="ps", name="pd")
            for _ in range(NWARM):
                nc.tensor.matmul(pd[:], wz_t[:], xz_t[:], start=True, stop=True)

            xten, xap0 = xs.tensor, list(xs.ap[0])
            wpten, wpap0 = wtp_t[:].tensor, list(wtp_t[:].ap[0])

            def rhs_pair(base, p, nb):
                off, stride = PAIR_GEOM[p]
                return bass.AP(xten, base + off, [xap0, [stride, 2], [1, nb]])

            def lhs_pair(p, h):
                return bass.AP(
                    wpten, p * 2 * COUT + h * 128, [wpap0, [COUT, 2], [1, 128]]
                )

            starts = [NB * t for t in range(len(NBS))]
            for h in range(NHALF):
                oq = nc.sync if h == 0 else nc.scalar
                for sg_i, (sg_start, sg_n) in enumerate(SG_HALF[h]):
                    tls = list(range(sg_start, sg_start + sg_n))
                    ow = sum(NBS[t] for t in tls)
                    ot = opool.tile([128, ow], dt.float8e4, tag="ot", name="ot")
                    ps = [
                        ppool.tile([128, NBS[t]], dt.float32, tag="ps", name="ps")
                        for t in tls
                    ]
                    for p in range(NPAIR):
                        wap = lhs_pair(p, h)
                        for j, t in enumerate(tls):
                            nc.tensor.matmul(
                                ps[j][:],
                                wap,
                                rhs_pair(GF + OBASE + starts[t], p, NBS[t]),
                                perf_mode=DR,
                                start=(p == 0),
                                stop=False,
                            )
                    wap = wts_t[:, h * 128 : (h + 1) * 128]
                    for j, t in enumerate(tls):
                        b = GF + OBASE + starts[t] + SINGLE_OFF
                        nc.tensor.matmul(
                            ps[j][:],
                            wap,
                            xs[:, b : b + NBS[t]],
                            start=False,
                            stop=True,
                        )
                    ob = 0
                    for j, t in enumerate(tls):
                        nc.vector.tensor_scalar(
                            out=ot[:, ob : ob + NBS[t]],
                            in0=ps[j][:],
                            scalar1=th_t[:, h : h + 1],
                            scalar2=None,
                            op0=mybir.AluOpType.is_ge,
                        )
                        ob += NBS[t]
                    dst = out[h][:, starts[sg_start] : starts[sg_start] + ow]
                    # keep every half-1 DMA (incl. the last) on the hot scalar
                    # ring — a cold ring pays the ~1.5us doorbell latency
                    oq.dma_start(out=dst, in_=ot[:])
    nc.finalize()
    return nc


def kernel(x, weight, bias, sign):
    global LAST_RESULT
    from concourse.bass_utils import run_bass_kernel_spmd

    if "nc" not in _CACHE:
        _CACHE["nc"] = _build()
    nc = _CACHE["nc"]

    sign_v = np.asarray(sign, dtype=np.float32).reshape(COUT)
    wsig = np.asarray(weight, dtype=np.float32) * sign_v[:, None, None, None]
    # wsig[cout, cin, kh, kw] -> pairs [cin, pair, 2, cout]
    wtp_host = np.zeros((CIN, NPAIR, 2, COUT), dtype=np.float32)
    for kw in range(KW):  # pairs 0..2: (kh0, kw), (kh1, kw)
        wtp_host[:, kw, 0] = wsig[:, :, 0, kw].T
        wtp_host[:, kw, 1] = wsig[:, :, 1, kw].T
    wtp_host[:, 3, 0] = wsig[:, :, 2, 0].T  # pair 3: (kh2,kw0),(kh2,kw2)
    wtp_host[:, 3, 1] = wsig[:, :, 2, 2].T
    wtp_host = wtp_host.reshape(CIN, NPAIR * 2 * COUT).astype(FP8)
    wts_host = np.ascontiguousarray(wsig[:, :, 2, 1].T).astype(FP8)
    th_host = np.ascontiguousarray(
        (-sign_v * np.asarray(bias, dtype=np.float32)).reshape(NHALF, 128).T
    ).astype(np.float32)

    x = np.asarray(x, dtype=np.float32)
    in_maps = []
    for c in range(N_CORES):
        xpad = np.zeros((CIN, ROWS_T, WS), dtype=FP8)
        for i in range(IMGS):
            r0 = 1 + i * (H + 1)
            xpad[:, r0 : r0 + H, :W] = x[c * IMGS + i]
        in_maps.append(
            {
                "xp": xpad.reshape(CIN, XD),
                "wtp": wtp_host,
                "wts": wts_host,
                "th": th_host,
            }
        )

    res = run_bass_kernel_spmd(nc, in_maps, core_ids=list(range(N_CORES)))
    LAST_RESULT = res
    # strip stored junk: out[h, co, j], j = (i*65 + r)*65 + c for valid r<64,
    # c<64 (junk at c=64 and in the 3 separator rows)
    full = np.empty((N, COUT, H, W), dtype=np.float32)
    pad1 = np.zeros((NHALF, 128, 1), dtype=FP8)
    for c, r in enumerate(res.results):
        v = np.concatenate([r["out"], pad1], axis=-1)
        v = v.reshape(NHALF, 128, ROWS_T - 2, WS)
        for i in range(IMGS):
            blk = v[:, :, i * (H + 1) : i * (H + 1) + H, :W]
            full[c * IMGS + i] = blk.reshape(COUT, H, W).astype(np.float32)
    return np.ascontiguousarray(full)


# revision 11
# speedup vs baseline: 1.1883x; 1.1883x over previous
"""BinConv2dEval Trainium2 kernel (fp8 DoubleRow, packed-65 layout).

y = conv2d(x, W, stride 1, pad 1) + bias ; out = (round(y) * sign >= 0) ? 1 : 0

All values are integers (x in {0,1}, W in {-1,0,1}, bias integer), so round()
is a no-op and everything is exact in fp8e4 matmuls with fp32 PSUM
accumulation. Folding: with s = sign[c] in {+-1},
    (conv + bias) * s >= 0   <=>   conv(x, s*W) >= -s*bias
so host-side we fold sign into the (still ternary) weights and compare each
output channel against a per-channel threshold with one DVE is_ge op.

Sharding: data-parallel over batch N=32 -> 4 images per core on 8 cores.
Weights/bias/sign are tiny and replicated.

Layout: width-65 rows ([64 data][0]) with SHARED zero rows between images:
row stream = [z, img0 r0..63, z, img1 r0..63, z, img2 ..., z, img3 ..., z]
= 261 rows x 65 = 16965 elems per partition. The single trailing zero col
doubles as both x(r,-1) of the next row and x(r,64) of its own, and each
separator zero row is both bottom pad of img i and top pad of img i+1. The
conv's 9 taps become pure element offsets (kh-1)*65 + (kw-1), and the
4-image output region is ONE contiguous stream of 16834 positions per cout
half (junk at stored col 64 and in separator rows; host strips ~2.7%).

Taps pair into fp8 DoubleRow matmuls (2 MACs/cell/cycle, contraction 256):
(kh=0,kw)+(kh=1,kw) at stride 65 for kw in 0..2, (kh=2,kw=0)+(kh=2,kw=2) at
stride 2; only (kh=2,kw=1) is a normal-rate matmul -> 5 passes per tile.
Per half: 33 PSUM tiles (32x512 + 450), weight-stationary subgroups of <=4
tiles; one DVE tensor_scalar(is_ge) per tile drains PSUM -> SBUF as 0/1
**fp8** (4x less output DMA than fp32; host upcasts). Half1 ends with 2/2/1
tile subgroups so the final DVE drains hide under the last matmuls.

Head: a few zero-weight warm-up matmuls sized to the DMA doorbell latency
(~1.7us) + first-chunk transfer, so the PE HAM clock-gate releases around
the time real compute starts; input x is chunked so tile 0 lands first.
"""

import numpy as np
import ml_dtypes

N, CIN, H, W = 32, 128, 64, 64
COUT, KH, KW = 256, 3, 3
N_CORES = 8
IMGS = N // N_CORES          # 4 images per core
WS = W + 1                   # 65: stored row width ([64 data][0])
ROWS_T = 1 + IMGS * (H + 1)  # 261 stored rows (shared separators)
XD = ROWS_T * WS             # 16965 elements per partition
GF = 16                      # guard zeros before the image block
XTOT = GF + XD
OBASE = WS                   # first output position in the stream (img0 r0 c0)
OUT_N = (1 + IMGS * (H + 1) - 2) * WS + (W - 1) - OBASE + 1  # 16834
NB = 512                     # full PSUM tile free dim (one bank)
NBS = [NB] * 32 + [OUT_N - 32 * NB]   # 33 tiles: 32x512 + 450
NHALF = COUT // 128          # 2 cout halves
NPAIR = 4                    # DoubleRow tap pairs per pass group
FP8 = ml_dtypes.float8_e4m3  # TRN float8e4; {-1,0,1} and {0,1} are exact
NWARM = 6                    # zero-weight PE warm-up matmuls (N=512, cold)

# weight-stationary spans (start tile, n tiles); half1 tapers so the last
# DVE drains overlap the final matmuls instead of trailing them
SG_HALF = (
    tuple((s, 4) for s in range(0, 32, 4)) + ((32, 1),),
    tuple((s, 4) for s in range(0, 28, 4)) + ((28, 2), (30, 2), (32, 1)),
)

# input x chunk boundaries (elements per partition): tiles0-3 | tiles4-15 |
# rest. Tile t reads [OBASE + 512t - 66, OBASE + 512t + NBS + 65]. All x
# chunks ride ONE ring (sync) in need order — the 16 DMA engines are shared
# between rings, so a big low-priority chunk on the other ring would starve
# the critical first tiles.
XCH = (2210, 8450, XD)

_CACHE = {}
LAST_RESULT = None           # BassKernelResults of the last run (for profiling)


def _build():
    import concourse.bass as bass
    import concourse.mybir as mybir
    from concourse import bacc
    from concourse.tile import TileContext

    dt = mybir.dt
    nc = bacc.Bacc()
    xp = nc.dram_tensor("xp", [128, XD], dt.float8e4, kind="ExternalInput")
    # pair weights: [cin, pair, 2, cout] flattened; pairs 0..2 = (kh0,kh1) per
    # kw, pair 3 = ((kh2,kw0),(kh2,kw2))
    wtp = nc.dram_tensor(
        "wtp", [128, NPAIR * 2 * COUT], dt.float8e4, kind="ExternalInput"
    )
    # the lone single tap (kh2,kw1): [cin, cout]
    wts = nc.dram_tensor("wts", [128, COUT], dt.float8e4, kind="ExternalInput")
    th = nc.dram_tensor("th", [128, NHALF], dt.float32, kind="ExternalInput")
    # ACT-engine drain bias: 32 - 64*th, so sigmoid(64*conv + thb) saturates
    # to exactly 1.0 (arg >= +32) / 0.0 (arg <= -32) for integer conv, th
    thb = nc.dram_tensor("thb", [128, NHALF], dt.float32, kind="ExternalInput")
    out = nc.dram_tensor(
        "out", [NHALF, 128, OUT_N], dt.float8e4, kind="ExternalOutput"
    )

    DR = mybir.MatmulPerfMode.DoubleRow
    # (pair rhs offset, pair stride) per DoubleRow pair index
    PAIR_GEOM = [(-66, WS), (-65, WS), (-64, WS), (64, 2)]
    SINGLE_OFF = WS  # (kh2, kw1)

    with TileContext(nc) as tc:
        with (
            tc.tile_pool(name="const", bufs=1) as cpool,
            tc.tile_pool(name="xin", bufs=1) as xpool,
            tc.tile_pool(name="psum", bufs=8, space="PSUM") as ppool,
            tc.tile_pool(name="outb", bufs=5) as opool,
        ):
            # warm-up operands first in gpsimd order so dummies start early
            wz_t = cpool.tile([128, 128], dt.float8e4, tag="wz")
            nc.gpsimd.memset(wz_t[:], 0)
            xz_t = cpool.tile([128, NB], dt.float8e4, tag="xz")
            nc.gpsimd.memset(xz_t[:], 0)

            xs_t = xpool.tile([128, XTOT], dt.float8e4, tag="xs")
            xs = xs_t[:]
            # front guard (junk reads at o=OBASE-66 must not hit fp8 NaNs)
            nc.gpsimd.memset(xs[:, :GF], 0)

            # sync HWDGE queue: weights then x chunks, strictly in need order
            # (pair-0 weights split out so the first matmul waits on 64KB only)
            wtp_t = cpool.tile([128, NPAIR * 2 * COUT], dt.float8e4, tag="wtp")
            nc.sync.dma_start(out=wtp_t[:, :NB], in_=wtp[:, :NB])
            lo = XCH[0]
            nc.sync.dma_start(out=xs[:, GF : GF + lo], in_=xp[:, :lo])
            nc.sync.dma_start(out=wtp_t[:, NB:], in_=wtp[:, NB:])
            for hi in XCH[1:]:
                nc.sync.dma_start(out=xs[:, GF + lo : GF + hi], in_=xp[:, lo:hi])
                lo = hi
            # scalar HWDGE queue: only the tiny single-tap weights + thresholds
            wts_t = cpool.tile([128, COUT], dt.float8e4, tag="wts")
            nc.scalar.dma_start(out=wts_t[:], in_=wts[:])
            th_t = cpool.tile([128, NHALF], dt.float32, tag="th")
            nc.scalar.dma_start(out=th_t[:], in_=th[:])
            thb_t = cpool.tile([128, NHALF], dt.float32, tag="thb")
            nc.scalar.dma_start(out=thb_t[:], in_=thb[:])

            # Warm the PE clock (HAM un-throttle needs ~3.4us of sustained
            # activity) with zero-weight matmuls on a zeroed scratch tile
            # while the input DMA doorbell+transfer is still in flight.
            pd = ppool.tile([128, NB], dt.float32, tag="ps", name="pd")
            for _ in range(NWARM):
                nc.tensor.matmul(pd[:], wz_t[:], xz_t[:], start=True, stop=True)

            xten, xap0 = xs.tensor, list(xs.ap[0])
            wpten, wpap0 = wtp_t[:].tensor, list(wtp_t[:].ap[0])

            def rhs_pair(base, p, nb):
                off, stride = PAIR_GEOM[p]
                return bass.AP(xten, base + off, [xap0, [stride, 2], [1, nb]])

            def lhs_pair(p, h):
                return bass.AP(
                    wpten, p * 2 * COUT + h * 128, [wpap0, [COUT, 2], [1, 128]]
                )

            starts = [NB * t for t in range(len(NBS))]
            for h in range(NHALF):
                oq = nc.sync if h == 0 else nc.scalar
                for sg_i, (sg_start, sg_n) in enumerate(SG_HALF[h]):
                    tls = list(range(sg_start, sg_start + sg_n))
                    ow = sum(NBS[t] for t in tls)
                    ot = opool.tile([128, ow], dt.float8e4, tag="ot", name="ot")
                    ps = [
                        ppool.tile([128, NBS[t]], dt.float32, tag="ps", name="ps")
                        for t in tls
                    ]
                    for p in range(NPAIR):
                        wap = lhs_pair(p, h)
                        for j, t in enumerate(tls):
                            nc.tensor.matmul(
                                ps[j][:],
                                wap,
                                rhs_pair(GF + OBASE + starts[t], p, NBS[t]),
                                perf_mode=DR,
                                start=(p == 0),
                                stop=False,
                            )
                    wap = wts_t[:, h * 128 : (h + 1) * 128]
                    for j, t in enumerate(tls):
                        b = GF + OBASE + starts[t] + SINGLE_OFF
                        nc.tensor.matmul(
                            ps[j][:],
                            wap,
                            xs[:, b : b + NBS[t]],
                            start=False,
                            stop=True,
                        )
                    ob = 0
                    for j, t in enumerate(tls):
                        if j % 2 == 0:
                            nc.vector.tensor_scalar(
                                out=ot[:, ob : ob + NBS[t]],
                                in0=ps[j][:],
                                scalar1=th_t[:, h : h + 1],
                                scalar2=None,
                                op0=mybir.AluOpType.is_ge,
                            )
                        else:
                            # exact on integers: arg is >= +32 or <= -32, where
                            # the sigmoid table saturates to exactly 1 / 0
                            nc.scalar.activation(
                                out=ot[:, ob : ob + NBS[t]],
                                in_=ps[j][:],
                                func=mybir.ActivationFunctionType.Sigmoid,
                                bias=thb_t[:, h : h + 1],
                                scale=64.0,
                            )
                        ob += NBS[t]
                    dst = out[h][:, starts[sg_start] : starts[sg_start] + ow]
                    # keep every half-1 DMA (incl. the last) on the hot scalar
                    # ring — a cold ring pays the ~1.5us doorbell latency
                    oq.dma_start(out=dst, in_=ot[:])
    nc.finalize()
    return nc


def kernel(x, weight, bias, sign):
    global LAST_RESULT
    from concourse.bass_utils import run_bass_kernel_spmd

    if "nc" not in _CACHE:
        _CACHE["nc"] = _build()
    nc = _CACHE["nc"]

    sign_v = np.asarray(sign, dtype=np.float32).reshape(COUT)
    wsig = np.asarray(weight, dtype=np.float32) * sign_v[:, None, None, None]
    # wsig[cout, cin, kh, kw] -> pairs [cin, pair, 2, cout]
    wtp_host = np.zeros((CIN, NPAIR, 2, COUT), dtype=np.float32)
    for kw in range(KW):  # pairs 0..2: (kh0, kw), (kh1, kw)
        wtp_host[:, kw, 0] = wsig[:, :, 0, kw].T
        wtp_host[:, kw, 1] = wsig[:, :, 1, kw].T
    wtp_host[:, 3, 0] = wsig[:, :, 2, 0].T  # pair 3: (kh2,kw0),(kh2,kw2)
    wtp_host[:, 3, 1] = wsig[:, :, 2, 2].T
    wtp_host = wtp_host.reshape(CIN, NPAIR * 2 * COUT).astype(FP8)
    wts_host = np.ascontiguousarray(wsig[:, :, 2, 1].T).astype(FP8)
    th_host = np.ascontiguousarray(
        (-sign_v * np.asarray(bias, dtype=np.float32)).reshape(NHALF, 128).T
    ).astype(np.float32)
    thb_host = (32.0 - 64.0 * th_host).astype(np.float32)

    x = np.asarray(x, dtype=np.float32)
    in_maps = []
    for c in range(N_CORES):
        xpad = np.zeros((CIN, ROWS_T, WS), dtype=FP8)
        for i in range(IMGS):
            r0 = 1 + i * (H + 1)
            xpad[:, r0 : r0 + H, :W] = x[c * IMGS + i]
        in_maps.append(
            {
                "xp": xpad.reshape(CIN, XD),
                "wtp": wtp_host,
                "wts": wts_host,
                "th": th_host,
                "thb": thb_host,
            }
        )

    res = run_bass_kernel_spmd(nc, in_maps, core_ids=list(range(N_CORES)))
    LAST_RESULT = res
    # strip stored junk: out[h, co, j], j = (i*65 + r)*65 + c for valid r<64,
    # c<64 (junk at c=64 and in the 3 separator rows)
    full = np.empty((N, COUT, H, W), dtype=np.float32)
    pad1 = np.zeros((NHALF, 128, 1), dtype=FP8)
    for c, r in enumerate(res.results):
        v = np.concatenate([r["out"], pad1], axis=-1)
        v = v.reshape(NHALF, 128, ROWS_T - 2, WS)
        for i in range(IMGS):
            blk = v[:, :, i * (H + 1) : i * (H + 1) + H, :W]
            full[c * IMGS + i] = blk.reshape(COUT, H, W).astype(np.float32)
    return np.ascontiguousarray(full)


# revision 15
# speedup vs baseline: 1.1992x; 1.0092x over previous
"""BinConv2dEval Trainium2 kernel (fp8 DoubleRow, packed-65 layout).

y = conv2d(x, W, stride 1, pad 1) + bias ; out = (round(y) * sign >= 0) ? 1 : 0

All values are integers (x in {0,1}, W in {-1,0,1}, bias integer), so round()
is a no-op and everything is exact in fp8e4 matmuls with fp32 PSUM
accumulation. Folding: with s = sign[c] in {+-1},
    (conv + bias) * s >= 0   <=>   conv(x, s*W) >= -s*bias
so host-side we fold sign into the (still ternary) weights and compare each
output channel against a per-channel threshold with one DVE is_ge op.

Sharding: data-parallel over batch N=32 -> 4 images per core on 8 cores.
Weights/bias/sign are tiny and replicated.

Layout: width-65 rows ([64 data][0]) with SHARED zero rows between images:
row stream = [z, img0 r0..63, z, img1 r0..63, z, img2 ..., z, img3 ..., z]
= 261 rows x 65 = 16965 elems per partition. The single trailing zero col
doubles as both x(r,-1) of the next row and x(r,64) of its own, and each
separator zero row is both bottom pad of img i and top pad of img i+1. The
conv's 9 taps become pure element offsets (kh-1)*65 + (kw-1), and the
4-image output region is ONE contiguous stream of 16834 positions per cout
half (junk at stored col 64 and in separator rows; host strips ~2.7%).

Taps pair into fp8 DoubleRow matmuls (2 MACs/cell/cycle, contraction 256):
(kh=0,kw)+(kh=1,kw) at stride 65 for kw in 0..2, (kh=2,kw=0)+(kh=2,kw=2) at
stride 2; only (kh=2,kw=1) is a normal-rate matmul -> 5 passes per tile.
Per half: 33 PSUM tiles (32x512 + 450), weight-stationary subgroups of <=4
tiles; one DVE tensor_scalar(is_ge) per tile drains PSUM -> SBUF as 0/1
**fp8** (4x less output DMA than fp32; host upcasts). Half1 ends with 2/2/1
tile subgroups so the final DVE drains hide under the last matmuls.

Head: a few zero-weight warm-up matmuls sized to the DMA doorbell latency
(~1.7us) + first-chunk transfer, so the PE HAM clock-gate releases around
the time real compute starts; input x is chunked so tile 0 lands first.
"""

import numpy as np
import ml_dtypes

N, CIN, H, W = 32, 128, 64, 64
COUT, KH, KW = 256, 3, 3
N_CORES = 8
IMGS = N // N_CORES          # 4 images per core
WS = W + 1                   # 65: stored row width ([64 data][0])
ROWS_T = 1 + IMGS * (H + 1)  # 261 stored rows (shared separators)
XD = ROWS_T * WS             # 16965 elements per partition
GF = 16                      # guard zeros before the image block
XTOT = GF + XD
OBASE = WS                   # first output position in the stream (img0 r0 c0)
OUT_N = (1 + IMGS * (H + 1) - 2) * WS + (W - 1) - OBASE + 1  # 16834
NB = 512                     # full PSUM tile free dim (one bank)
NBS = [NB] * 32 + [OUT_N - 32 * NB]   # 33 tiles: 32x512 + 450
NHALF = COUT // 128          # 2 cout halves
NPAIR = 4                    # DoubleRow tap pairs per pass group
FP8 = ml_dtypes.float8_e4m3  # TRN float8e4; {-1,0,1} and {0,1} are exact
NWARM = 5                    # zero-weight PE warm-up matmuls (N=512, cold)

# weight-stationary spans (start tile, n tiles); half0 ramps up in 2-tile
# spans so compute can start on a smaller first DMA chunk, half1 tapers so
# the last drains overlap the final matmuls instead of trailing them
SG_HALF = (
    ((0, 2), (2, 2)) + tuple((s, 4) for s in range(4, 32, 4)) + ((32, 1),),
    tuple((s, 4) for s in range(0, 28, 4)) + ((28, 2), (30, 2), (32, 1)),
)

# input x chunk boundaries (elements per partition): tiles0-1 | tiles2-3 |
# tiles4-15 | rest. Tile t reads [OBASE + 512t - 66, OBASE + 512t + NBS + 65].
# All x chunks ride ONE ring (sync) in need order — the 16 DMA engines are
# shared between rings, so a big low-priority chunk on the other ring would
# starve the critical first tiles.
XCH = (1170, 2210, 8450, XD)

_CACHE = {}
LAST_RESULT = None           # BassKernelResults of the last run (for profiling)


def _build():
    import concourse.bass as bass
    import concourse.mybir as mybir
    from concourse import bacc
    from concourse.tile import TileContext

    dt = mybir.dt
    nc = bacc.Bacc()
    xp = nc.dram_tensor("xp", [128, XD], dt.float8e4, kind="ExternalInput")
    # pair weights: [cin, pair, 2, cout] flattened; pairs 0..2 = (kh0,kh1) per
    # kw, pair 3 = ((kh2,kw0),(kh2,kw2))
    wtp = nc.dram_tensor(
        "wtp", [128, NPAIR * 2 * COUT], dt.float8e4, kind="ExternalInput"
    )
    # the lone single tap (kh2,kw1): [cin, cout]
    wts = nc.dram_tensor("wts", [128, COUT], dt.float8e4, kind="ExternalInput")
    th = nc.dram_tensor("th", [128, NHALF], dt.float32, kind="ExternalInput")
    # ACT-engine drain bias: 32 - 64*th, so sigmoid(64*conv + thb) saturates
    # to exactly 1.0 (arg >= +32) / 0.0 (arg <= -32) for integer conv, th
    thb = nc.dram_tensor("thb", [128, NHALF], dt.float32, kind="ExternalInput")
    out = nc.dram_tensor(
        "out", [NHALF, 128, OUT_N], dt.float8e4, kind="ExternalOutput"
    )

    DR = mybir.MatmulPerfMode.DoubleRow
    # (pair rhs offset, pair stride) per DoubleRow pair index
    PAIR_GEOM = [(-66, WS), (-65, WS), (-64, WS), (64, 2)]
    SINGLE_OFF = WS  # (kh2, kw1)

    with TileContext(nc) as tc:
        with (
            tc.tile_pool(name="const", bufs=1) as cpool,
            tc.tile_pool(name="xin", bufs=1) as xpool,
            tc.tile_pool(name="psum", bufs=8, space="PSUM") as ppool,
            tc.tile_pool(name="outb", bufs=5) as opool,
        ):
            # warm-up operands first in gpsimd order so dummies start early
            wz_t = cpool.tile([128, 128], dt.float8e4, tag="wz")
            nc.gpsimd.memset(wz_t[:], 0)
            xz_t = cpool.tile([128, NB], dt.float8e4, tag="xz")
            nc.gpsimd.memset(xz_t[:], 0)

            xs_t = xpool.tile([128, XTOT], dt.float8e4, tag="xs")
            xs = xs_t[:]
            # front guard (junk reads at o=OBASE-66 must not hit fp8 NaNs)
            nc.gpsimd.memset(xs[:, :GF], 0)

            # sync HWDGE queue: weights then x chunks, strictly in need order
            # (pair-0 weights split out so the first matmul waits on 64KB only)
            wtp_t = cpool.tile([128, NPAIR * 2 * COUT], dt.float8e4, tag="wtp")
            nc.sync.dma_start(out=wtp_t[:, :NB], in_=wtp[:, :NB])
            lo = XCH[0]
            nc.sync.dma_start(out=xs[:, GF : GF + lo], in_=xp[:, :lo])
            nc.sync.dma_start(
                out=xs[:, GF + lo : GF + XCH[1]], in_=xp[:, lo : XCH[1]]
            )
            nc.sync.dma_start(out=wtp_t[:, NB:], in_=wtp[:, NB:])
            lo = XCH[1]
            for hi in XCH[2:]:
                nc.sync.dma_start(out=xs[:, GF + lo : GF + hi], in_=xp[:, lo:hi])
                lo = hi
            # scalar HWDGE queue: only the tiny single-tap weights + thresholds
            wts_t = cpool.tile([128, COUT], dt.float8e4, tag="wts")
            nc.scalar.dma_start(out=wts_t[:], in_=wts[:])
            th_t = cpool.tile([128, NHALF], dt.float32, tag="th")
            nc.scalar.dma_start(out=th_t[:], in_=th[:])
            thb_t = cpool.tile([128, NHALF], dt.float32, tag="thb")
            nc.scalar.dma_start(out=thb_t[:], in_=thb[:])

            # Warm the PE clock (HAM un-throttle needs ~3.4us of sustained
            # activity) with zero-weight matmuls on a zeroed scratch tile
            # while the input DMA doorbell+transfer is still in flight.
            pd = ppool.tile([128, NB], dt.float32, tag="ps", name="pd")
            for _ in range(NWARM):
                nc.tensor.matmul(pd[:], wz_t[:], xz_t[:], start=True, stop=True)

            xten, xap0 = xs.tensor, list(xs.ap[0])
            wpten, wpap0 = wtp_t[:].tensor, list(wtp_t[:].ap[0])

            def rhs_pair(base, p, nb):
                off, stride = PAIR_GEOM[p]
                return bass.AP(xten, base + off, [xap0, [stride, 2], [1, nb]])

            def lhs_pair(p, h):
                return bass.AP(
                    wpten, p * 2 * COUT + h * 128, [wpap0, [COUT, 2], [1, 128]]
                )

            starts = [NB * t for t in range(len(NBS))]
            flip = False
            for h in range(NHALF):
                oq = nc.sync if h == 0 else nc.scalar
                for sg_i, (sg_start, sg_n) in enumerate(SG_HALF[h]):
                    tls = list(range(sg_start, sg_start + sg_n))
                    ow = sum(NBS[t] for t in tls)
                    ot = opool.tile([128, ow], dt.float8e4, tag="ot", name="ot")
                    ps = [
                        ppool.tile([128, NBS[t]], dt.float32, tag="ps", name="ps")
                        for t in tls
                    ]
                    passes = [0, 1, 2, 3, 4]
                    for k, p in enumerate(passes):
                        st, sp = k == 0, k == NPAIR
                        if p < NPAIR:
                            wap = lhs_pair(p, h)
                            for j, t in enumerate(tls):
                                nc.tensor.matmul(
                                    ps[j][:],
                                    wap,
                                    rhs_pair(GF + OBASE + starts[t], p, NBS[t]),
                                    perf_mode=DR,
                                    start=st,
                                    stop=sp,
                                )
                        else:
                            wap = wts_t[:, h * 128 : (h + 1) * 128]
                            for j, t in enumerate(tls):
                                b = GF + OBASE + starts[t] + SINGLE_OFF
                                nc.tensor.matmul(
                                    ps[j][:],
                                    wap,
                                    xs[:, b : b + NBS[t]],
                                    start=st,
                                    stop=sp,
                                )
                    ob = 0
                    for j, t in enumerate(tls):
                        if j % 2 == 0:
                            nc.vector.tensor_scalar(
                                out=ot[:, ob : ob + NBS[t]],
                                in0=ps[j][:],
                                scalar1=th_t[:, h : h + 1],
                                scalar2=None,
                                op0=mybir.AluOpType.is_ge,
                            )
                        else:
                            # exact on integers: arg is >= +32 or <= -32, where
                            # the sigmoid table saturates to exactly 1 / 0
                            nc.scalar.activation(
                                out=ot[:, ob : ob + NBS[t]],
                                in_=ps[j][:],
                                func=mybir.ActivationFunctionType.Sigmoid,
                                bias=thb_t[:, h : h + 1],
                                scale=64.0,
                            )
                        ob += NBS[t]
                    dst = out[h][:, starts[sg_start] : starts[sg_start] + ow]
                    # keep every half-1 DMA (incl. the last) on the hot scalar
                    # ring — a cold ring pays the ~1.5us doorbell latency
                    oq.dma_start(out=dst, in_=ot[:])
    nc.finalize()
    return nc


def kernel(x, weight, bias, sign):
    global LAST_RESULT
    from concourse.bass_utils import run_bass_kernel_spmd

    if "nc" not in _CACHE:
        _CACHE["nc"] = _build()
    nc = _CACHE["nc"]

    sign_v = np.asarray(sign, dtype=np.float32).reshape(COUT)
    wsig = np.asarray(weight, dtype=np.float32) * sign_v[:, None, None, None]
    # wsig[cout, cin, kh, kw] -> pairs [cin, pair, 2, cout]
    wtp_host = np.zeros((CIN, NPAIR, 2, COUT), dtype=np.float32)
    for kw in range(KW):  # pairs 0..2: (kh0, kw), (kh1, kw)
        wtp_host[:, kw, 0] = wsig[:, :, 0, kw].T
        wtp_host[:, kw, 1] = wsig[:, :, 1, kw].T
    wtp_host[:, 3, 0] = wsig[:, :, 2, 0].T  # pair 3: (kh2,kw0),(kh2,kw2)
    wtp_host[:, 3, 1] = wsig[:, :, 2, 2].T
    wtp_host = wtp_host.reshape(CIN, NPAIR * 2 * COUT).astype(FP8)
    wts_host = np.ascontiguousarray(wsig[:, :, 2, 1].T).astype(FP8)
    th_host = np.ascontiguousarray(
        (-sign_v * np.asarray(bias, dtype=np.float32)).reshape(NHALF, 128).T
    ).astype(np.float32)
    thb_host = (32.0 - 64.0 * th_host).astype(np.float32)

    x = np.asarray(x, dtype=np.float32)
    in_maps = []
    for c in range(N_CORES):
        xpad = np.zeros((CIN, ROWS_T, WS), dtype=FP8)
        for i in range(IMGS):
            r0 = 1 + i * (H + 1)
            xpad[:, r0 : r0 + H, :W] = x[c * IMGS + i]
        in_maps.append(
            {
                "xp": xpad.reshape(CIN, XD),
                "wtp": wtp_host,
                "wts": wts_host,
                "th": th_host,
                "thb": thb_host,
            }
        )

    res = run_bass_kernel_spmd(nc, in_maps, core_ids=list(range(N_CORES)))
    LAST_RESULT = res
    # strip stored junk: out[h, co, j], j = (i*65 + r)*65 + c for valid r<64,
    # c<64 (junk at c=64 and in the 3 separator rows)
    full = np.empty((N, COUT, H, W), dtype=np.float32)
    pad1 = np.zeros((NHALF, 128, 1), dtype=FP8)
    for c, r in enumerate(res.results):
        v = np.concatenate([r["out"], pad1], axis=-1)
        v = v.reshape(NHALF, 128, ROWS_T - 2, WS)
        for i in range(IMGS):
            blk = v[:, :, i * (H + 1) : i * (H + 1) + H, :W]
            full[c * IMGS + i] = blk.reshape(COUT, H, W).astype(np.float32)
    return np.ascontiguousarray(full)


# revision 16
# speedup vs baseline: 1.2153x; 1.0134x over previous
"""BinConv2dEval Trainium2 kernel (fp8 DoubleRow, packed-65 layout).

y = conv2d(x, W, stride 1, pad 1) + bias ; out = (round(y) * sign >= 0) ? 1 : 0

All values are integers (x in {0,1}, W in {-1,0,1}, bias integer), so round()
is a no-op and everything is exact in fp8e4 matmuls with fp32 PSUM
accumulation. Folding: with s = sign[c] in {+-1},
    (conv + bias) * s >= 0   <=>   conv(x, s*W) >= -s*bias
so host-side we fold sign into the (still ternary) weights and compare each
output channel against a per-channel threshold with one DVE is_ge op.

Sharding: data-parallel over batch N=32 -> 4 images per core on 8 cores.
Weights/bias/sign are tiny and replicated.

Layout: width-65 rows ([64 data][0]) with SHARED zero rows between images:
row stream = [z, img0 r0..63, z, img1 r0..63, z, img2 ..., z, img3 ..., z]
= 261 rows x 65 = 16965 elems per partition. The single trailing zero col
doubles as both x(r,-1) of the next row and x(r,64) of its own, and each
separator zero row is both bottom pad of img i and top pad of img i+1. The
conv's 9 taps become pure element offsets (kh-1)*65 + (kw-1), and the
4-image output region is ONE contiguous stream of 16834 positions per cout
half (junk at stored col 64 and in separator rows; host strips ~2.7%).

Taps pair into fp8 DoubleRow matmuls (2 MACs/cell/cycle, contraction 256):
(kh=0,kw)+(kh=1,kw) at stride 65 for kw in 0..2, (kh=2,kw=0)+(kh=2,kw=2) at
stride 2; only (kh=2,kw=1) is a normal-rate matmul -> 5 passes per tile.
Per half: 33 PSUM tiles (32x512 + 450), weight-stationary subgroups of <=4
tiles; one DVE tensor_scalar(is_ge) per tile drains PSUM -> SBUF as 0/1
**fp8** (4x less output DMA than fp32; host upcasts). Half1 ends with 2/2/1
tile subgroups so the final DVE drains hide under the last matmuls.

Head: a few zero-weight warm-up matmuls sized to the DMA doorbell latency
(~1.7us) + first-chunk transfer, so the PE HAM clock-gate releases around
the time real compute starts; input x is chunked so tile 0 lands first.
"""

import numpy as np
import ml_dtypes

N, CIN, H, W = 32, 128, 64, 64
COUT, KH, KW = 256, 3, 3
N_CORES = 8
IMGS = N // N_CORES          # 4 images per core
WS = W + 1                   # 65: stored row width ([64 data][0])
ROWS_T = 1 + IMGS * (H + 1)  # 261 stored rows (shared separators)
XD = ROWS_T * WS             # 16965 elements per partition
GF = 16                      # guard zeros before the image block
XTOT = GF + XD
OBASE = WS                   # first output position in the stream (img0 r0 c0)
OUT_N = (1 + IMGS * (H + 1) - 2) * WS + (W - 1) - OBASE + 1  # 16834
NB = 512                     # full PSUM tile free dim (one bank)
NBS = [NB] * 32 + [OUT_N - 32 * NB]   # 33 tiles: 32x512 + 450
NHALF = COUT // 128          # 2 cout halves
NPAIR = 4                    # DoubleRow tap pairs per pass group
FP8 = ml_dtypes.float8_e4m3  # TRN float8e4; {-1,0,1} and {0,1} are exact
NWARM = 8                    # zero-weight PE warm-up matmuls (N=512, cold)

# weight-stationary spans (start tile, n tiles); half0 ramps up in 2-tile
# spans so compute can start on a smaller first DMA chunk, 6-tile spans in
# the middle (drains split over DVE+ACT keep up) to cut boundary stalls,
# half1 tapers so the last drains overlap the final matmuls
SG_HALF = (
    ((0, 2), (2, 2), (4, 6), (10, 6), (16, 6), (22, 6), (28, 4), (32, 1)),
    ((0, 6), (6, 6), (12, 6), (18, 6), (24, 4), (28, 2), (30, 2), (32, 1)),
)

# input x chunk boundaries (elements per partition): tiles0-1 | tiles2-3 |
# tiles4-15 | rest. Tile t reads [OBASE + 512t - 66, OBASE + 512t + NBS + 65].
# All x chunks ride ONE ring (sync) in need order — the 16 DMA engines are
# shared between rings, so a big low-priority chunk on the other ring would
# starve the critical first tiles.
XCH = (1170, 2210, 8450, XD)

_CACHE = {}
LAST_RESULT = None           # BassKernelResults of the last run (for profiling)


def _build():
    import concourse.bass as bass
    import concourse.mybir as mybir
    from concourse import bacc
    from concourse.tile import TileContext

    dt = mybir.dt
    nc = bacc.Bacc()
    xp = nc.dram_tensor("xp", [128, XD], dt.float8e4, kind="ExternalInput")
    # pair weights: [cin, pair, 2, cout] flattened; pairs 0..2 = (kh0,kh1) per
    # kw, pair 3 = ((kh2,kw0),(kh2,kw2))
    wtp = nc.dram_tensor(
        "wtp", [128, NPAIR * 2 * COUT], dt.float8e4, kind="ExternalInput"
    )
    # the lone single tap (kh2,kw1): [cin, cout]
    wts = nc.dram_tensor("wts", [128, COUT], dt.float8e4, kind="ExternalInput")
    th = nc.dram_tensor("th", [128, NHALF], dt.float32, kind="ExternalInput")
    # ACT-engine drain bias: 32 - 64*th, so sigmoid(64*conv + thb) saturates
    # to exactly 1.0 (arg >= +32) / 0.0 (arg <= -32) for integer conv, th
    thb = nc.dram_tensor("thb", [128, NHALF], dt.float32, kind="ExternalInput")
    out = nc.dram_tensor(
        "out", [NHALF, 128, OUT_N], dt.float8e4, kind="ExternalOutput"
    )

    DR = mybir.MatmulPerfMode.DoubleRow
    # (pair rhs offset, pair stride) per DoubleRow pair index
    PAIR_GEOM = [(-66, WS), (-65, WS), (-64, WS), (64, 2)]
    SINGLE_OFF = WS  # (kh2, kw1)

    with TileContext(nc) as tc:
        with (
            tc.tile_pool(name="const", bufs=1) as cpool,
            tc.tile_pool(name="xin", bufs=1) as xpool,
            tc.tile_pool(name="psum", bufs=8, space="PSUM") as ppool,
            tc.tile_pool(name="outb", bufs=5) as opool,
        ):
            # warm-up operands first in gpsimd order so dummies start early
            wz_t = cpool.tile([128, 128], dt.float8e4, tag="wz")
            nc.gpsimd.memset(wz_t[:], 0)
            xz_t = cpool.tile([128, NB], dt.float8e4, tag="xz")
            nc.gpsimd.memset(xz_t[:], 0)

            xs_t = xpool.tile([128, XTOT], dt.float8e4, tag="xs")
            xs = xs_t[:]
            # front guard (junk reads at o=OBASE-66 must not hit fp8 NaNs)
            nc.gpsimd.memset(xs[:, :GF], 0)

            # sync HWDGE queue: weights then x chunks, strictly in need order
            # (pair-0 weights split out so the first matmul waits on 64KB only)
            wtp_t = cpool.tile([128, NPAIR * 2 * COUT], dt.float8e4, tag="wtp")
            nc.sync.dma_start(out=wtp_t[:, :NB], in_=wtp[:, :NB])
            lo = XCH[0]
            nc.sync.dma_start(out=xs[:, GF : GF + lo], in_=xp[:, :lo])
            nc.sync.dma_start(
                out=xs[:, GF + lo : GF + XCH[1]], in_=xp[:, lo : XCH[1]]
            )
            nc.sync.dma_start(out=wtp_t[:, NB:], in_=wtp[:, NB:])
            lo = XCH[1]
            for hi in XCH[2:]:
                nc.sync.dma_start(out=xs[:, GF + lo : GF + hi], in_=xp[:, lo:hi])
                lo = hi
            # scalar HWDGE queue: only the tiny single-tap weights + thresholds
            wts_t = cpool.tile([128, COUT], dt.float8e4, tag="wts")
            nc.scalar.dma_start(out=wts_t[:], in_=wts[:])
            th_t = cpool.tile([128, NHALF], dt.float32, tag="th")
            nc.scalar.dma_start(out=th_t[:], in_=th[:])
            thb_t = cpool.tile([128, NHALF], dt.float32, tag="thb")
            nc.scalar.dma_start(out=thb_t[:], in_=thb[:])

            # Warm the PE clock (HAM un-throttle needs ~3.4us of sustained
            # activity) with zero-weight matmuls on a zeroed scratch tile
            # while the input DMA doorbell+transfer is still in flight.
            pd = ppool.tile([128, NB], dt.float32, tag="ps", name="pd")
            for _ in range(NWARM):
                nc.tensor.matmul(pd[:], wz_t[:], xz_t[:], start=True, stop=True)

            xten, xap0 = xs.tensor, list(xs.ap[0])
            wpten, wpap0 = wtp_t[:].tensor, list(wtp_t[:].ap[0])

            def rhs_pair(base, p, nb):
                off, stride = PAIR_GEOM[p]
                return bass.AP(xten, base + off, [xap0, [stride, 2], [1, nb]])

            def lhs_pair(p, h):
                return bass.AP(
                    wpten, p * 2 * COUT + h * 128, [wpap0, [COUT, 2], [1, 128]]
                )

            starts = [NB * t for t in range(len(NBS))]
            flip = False
            for h in range(NHALF):
                oq = nc.sync if h == 0 else nc.scalar
                for sg_i, (sg_start, sg_n) in enumerate(SG_HALF[h]):
                    tls = list(range(sg_start, sg_start + sg_n))
                    ow = sum(NBS[t] for t in tls)
                    ot = opool.tile([128, ow], dt.float8e4, tag="ot", name="ot")
                    ps = [
                        ppool.tile([128, NBS[t]], dt.float32, tag="ps", name="ps")
                        for t in tls
                    ]
                    passes = [0, 1, 2, 3, 4]
                    for k, p in enumerate(passes):
                        st, sp = k == 0, k == NPAIR
                        if p < NPAIR:
                            wap = lhs_pair(p, h)
                            for j, t in enumerate(tls):
                                nc.tensor.matmul(
                                    ps[j][:],
                                    wap,
                                    rhs_pair(GF + OBASE + starts[t], p, NBS[t]),
                                    perf_mode=DR,
                                    start=st,
                                    stop=sp,
                                )
                        else:
                            wap = wts_t[:, h * 128 : (h + 1) * 128]
                            for j, t in enumerate(tls):
                                b = GF + OBASE + starts[t] + SINGLE_OFF
                                nc.tensor.matmul(
                                    ps[j][:],
                                    wap,
                                    xs[:, b : b + NBS[t]],
                                    start=st,
                                    stop=sp,
                                )
                    ob = 0
                    for j, t in enumerate(tls):
                        if j % 2 == 0:
                            nc.vector.tensor_scalar(
                                out=ot[:, ob : ob + NBS[t]],
                                in0=ps[j][:],
                                scalar1=th_t[:, h : h + 1],
                                scalar2=None,
                                op0=mybir.AluOpType.is_ge,
                            )
                        else:
                            # exact on integers: arg is >= +32 or <= -32, where
                            # the sigmoid table saturates to exactly 1 / 0
                            nc.scalar.activation(
                                out=ot[:, ob : ob + NBS[t]],
                                in_=ps[j][:],
                                func=mybir.ActivationFunctionType.Sigmoid,
                                bias=thb_t[:, h : h + 1],
                                scale=64.0,
                            )
                        ob += NBS[t]
                    dst = out[h][:, starts[sg_start] : starts[sg_start] + ow]
                    # keep every half-1 DMA (incl. the last) on the hot scalar
                    # ring — a cold ring pays the ~1.5us doorbell latency
                    oq.dma_start(out=dst, in_=ot[:])
    nc.finalize()
    return nc


def kernel(x, weight, bias, sign):
    global LAST_RESULT
    from concourse.bass_utils import run_bass_kernel_spmd

    if "nc" not in _CACHE:
        _CACHE["nc"] = _build()
    nc = _CACHE["nc"]

    sign_v = np.asarray(sign, dtype=np.float32).reshape(COUT)
    wsig = np.asarray(weight, dtype=np.float32) * sign_v[:, None, None, None]
    # wsig[cout, cin, kh, kw] -> pairs [cin, pair, 2, cout]
    wtp_host = np.zeros((CIN, NPAIR, 2, COUT), dtype=np.float32)
    for kw in range(KW):  # pairs 0..2: (kh0, kw), (kh1, kw)
        wtp_host[:, kw, 0] = wsig[:, :, 0, kw].T
        wtp_host[:, kw, 1] = wsig[:, :, 1, kw].T
    wtp_host[:, 3, 0] = wsig[:, :, 2, 0].T  # pair 3: (kh2,kw0),(kh2,kw2)
    wtp_host[:, 3, 1] = wsig[:, :, 2, 2].T
    wtp_host = wtp_host.reshape(CIN, NPAIR * 2 * COUT).astype(FP8)
    wts_host = np.ascontiguousarray(wsig[:, :, 2, 1].T).astype(FP8)
    th_host = np.ascontiguousarray(
        (-sign_v * np.asarray(bias, dtype=np.float32)).reshape(NHALF, 128).T
    ).astype(np.float32)
    thb_host = (32.0 - 64.0 * th_host).astype(np.float32)

    x = np.asarray(x, dtype=np.float32)
    in_maps = []
    for c in range(N_CORES):
        xpad = np.zeros((CIN, ROWS_T, WS), dtype=FP8)
        for i in range(IMGS):
            r0 = 1 + i * (H + 1)
            xpad[:, r0 : r0 + H, :W] = x[c * IMGS + i]
        in_maps.append(
            {
                "xp": xpad.reshape(CIN, XD),
                "wtp": wtp_host,
                "wts": wts_host,
                "th": th_host,
                "thb": thb_host,
            }
        )

    res = run_bass_kernel_spmd(nc, in_maps, core_ids=list(range(N_CORES)))
    LAST_RESULT = res
    # strip stored junk: out[h, co, j], j = (i*65 + r)*65 + c for valid r<64,
    # c<64 (junk at c=64 and in the 3 separator rows)
    full = np.empty((N, COUT, H, W), dtype=np.float32)
    pad1 = np.zeros((NHALF, 128, 1), dtype=FP8)
    for c, r in enumerate(res.results):
        v = np.concatenate([r["out"], pad1], axis=-1)
        v = v.reshape(NHALF, 128, ROWS_T - 2, WS)
        for i in range(IMGS):
            blk = v[:, :, i * (H + 1) : i * (H + 1) + H, :W]
            full[c * IMGS + i] = blk.reshape(COUT, H, W).astype(np.float32)
    return np.ascontiguousarray(full)
